# revision 18
# baseline (speedup 1.0000x reference)
"""Trainium2 Bass kernel for nn_LogicLayer (differentiable logic-gate layer).

Math:
    a = x[:, idx_a]; b = x[:, idx_b]                  # gather columns
    c = softmax(weights) @ T                          # [O, 4] truth-table coeffs
    out = c0*(1-a)(1-b) + c1*(1-a)b + c2*a(1-b) + c3*ab
        = k0 + ka*a + kb*b + kab*a*b
  with k0 = c0, ka = c2-c0, kb = c1-c0, kab = c0-c1-c2+c3.

Device strategy (8 cores, out_dim sharded, 2048 gates/core):
  - Host quantizes x to centered int16 fixed point u = rint((x-.5)*65536)
    (x = u/65536 + 1/2, abs err <= 2^-17), and pre-transposes to
    uT [in_dim, B] so a gate's input column is a contiguous 8KB row. The
    bilinear form stays bilinear in u with host-folded coefficients:
      out = K0 + KA*ua + KB*ub + KAB*ua*ub
      K0 = k0+ka/2+kb/2+kab/4, KA = (ka+kab/2)/65536,
      KB = (kb+kab/2)/65536,   KAB = kab/65536^2.
    This halves gather DMA vs f32 while keeping output error ~1e-5 abs.
  - Per 128-gate block: indirect-DMA gather the 128 a-columns and b-columns
    of uT into SBUF as [128 gates, 4096 batch] int16 tiles; coefficients are
    per-partition scalars: t = KAB*ua+KB (DVE tensor_scalar, int16->f32),
    v = KA*ua+K0 (ScalarE activation), t *= ub, t += v (DVE tensor_tensor);
    PE transposes 128x128 blocks back to [batch, gate] via PSUM, and
    strided-AP DMAs write the [4096, 2048] f32 output shard.
"""

import contextlib

import numpy as np

import concourse.bass as bass
import concourse.tile as tile
from concourse import bacc, mybir
from concourse.bass_utils import run_bass_kernel_spmd
from concourse.masks import make_identity

B = 4096          # batch
IN_DIM = 4096     # input features
O = 16384         # gates (out_dim)
NCORES = 8
OSH = O // NCORES  # 2048 gates per core
P = 128
GBLOCKS = OSH // P  # 16 gate blocks per core

USE_I16 = True    # gather int16 fixed-point x (2x less DMA) vs f32

_PROGRAMS = {}


def _build_program(reps=1, use_i16=USE_I16):
    f32 = mybir.dt.float32
    i32 = mybir.dt.int32
    xdt = mybir.dt.int16 if use_i16 else f32

    nc = bacc.Bacc(None)
    xt_d = nc.declare_dram_parameter("xt", [IN_DIM, B], xdt, isOutput=False)
    # per-core index/coef tensors pre-swizzled on host to [128, blocks*...]
    ia_d = nc.declare_dram_parameter("idxa", [P, GBLOCKS], i32, isOutput=False)
    ib_d = nc.declare_dram_parameter("idxb", [P, GBLOCKS], i32, isOutput=False)
    coef_d = nc.declare_dram_parameter("coef", [P, GBLOCKS * 4], f32, isOutput=False)
    out_d = nc.declare_dram_parameter("out", [B, OSH], f32, isOutput=True)
    # view: [batch_block j, 128 p, gate] -> [p, j, gate] for strided stores
    out_v = out_d[:].rearrange("(j p) g -> p j g", p=P)

    with tile.TileContext(nc) as tc:
        with (
            tc.tile_pool(name="const", bufs=1) as const_pool,
            tc.tile_pool(name="gath", bufs=4) as gath_pool,
            tc.tile_pool(name="tmp", bufs=3) as tmp_pool,
            tc.tile_pool(name="ot", bufs=8) as ot_pool,
            tc.tile_pool(name="psum", bufs=4, space="PSUM") as psum_pool,
        ):
            identity_t = const_pool.tile([P, P], f32)
            make_identity(nc, identity_t[:])

            idxa_t = const_pool.tile([P, GBLOCKS], i32)
            nc.sync.dma_start(out=idxa_t[:], in_=ia_d[:])
            idxb_t = const_pool.tile([P, GBLOCKS], i32)
            nc.sync.dma_start(out=idxb_t[:], in_=ib_d[:])
            coef_t = const_pool.tile([P, GBLOCKS * 4], f32)
            nc.sync.dma_start(out=coef_t[:], in_=coef_d[:])

            loop_cm = (
                tc.For_i(0, reps, 1) if reps > 1 else contextlib.nullcontext()
            )
            with loop_cm:
                for gb in range(GBLOCKS):
                    gsl = slice(gb * P, (gb + 1) * P)

                    a_t = gath_pool.tile([P, B], xdt, tag="a")
                    nc.gpsimd.indirect_dma_start(
                        out=a_t[:],
                        out_offset=None,
                        in_=xt_d[:],
                        in_offset=bass.IndirectOffsetOnAxis(
                            ap=idxa_t[:, gb : gb + 1], axis=0
                        ),
                    )
                    b_t = gath_pool.tile([P, B], xdt, tag="b")
                    nc.gpsimd.indirect_dma_start(
                        out=b_t[:],
                        out_offset=None,
                        in_=xt_d[:],
                        in_offset=bass.IndirectOffsetOnAxis(
                            ap=idxb_t[:, gb : gb + 1], axis=0
                        ),
                    )

                    # t = KAB*ua + KB   (per-partition scalars)
                    t_t = tmp_pool.tile([P, B], f32, tag="t")
                    nc.vector.tensor_scalar(
                        t_t[:],
                        a_t[:],
                        coef_t[:, 4 * gb + 3 : 4 * gb + 4],
                        coef_t[:, 4 * gb + 2 : 4 * gb + 3],
                        op0=mybir.AluOpType.mult,
                        op1=mybir.AluOpType.add,
                    )
                    # v = KA*ua + K0    (scalar engine)
                    v_t = tmp_pool.tile([P, B], f32, tag="v")
                    nc.scalar.activation(
                        v_t[:],
                        a_t[:],
                        mybir.ActivationFunctionType.Identity,
                        bias=coef_t[:, 4 * gb : 4 * gb + 1],
                        scale=coef_t[:, 4 * gb + 1 : 4 * gb + 2],
                    )
                    # t = t*ub  ->  product term [128 gates, B]; the +v is
                    # folded into the PE transposes via PSUM accumulation.
                    nc.vector.tensor_tensor(
                        out=t_t[:], in0=t_t[:], in1=b_t[:], op=mybir.AluOpType.mult
                    )

                    # Transpose back to [batch, gate] in 128x128 blocks, 8 per
                    # [128, 1024] PSUM tile (2 banks); each block accumulates
                    # t^T + v^T in PSUM (two is_transpose matmuls), then one
                    # copy -> SBUF and one strided-AP DMA covering 8 batch
                    # blocks.
                    for q in range(B // 1024):
                        ps = psum_pool.tile([P, 1024], f32, space="PSUM", tag="ps")
                        for j in range(8):
                            col = q * 1024 + j * P
                            nc.tensor.matmul(
                                out=ps[:, j * P : (j + 1) * P],
                                lhsT=t_t[:, col : col + P],
                                rhs=identity_t[:],
                                is_transpose=True,
                                start=True,
                                stop=False,
                            )
                            nc.tensor.matmul(
                                out=ps[:, j * P : (j + 1) * P],
                                lhsT=v_t[:, col : col + P],
                                rhs=identity_t[:],
                                is_transpose=True,
                                start=False,
                                stop=True,
                            )
                        sb = ot_pool.tile([P, 1024], f32, tag="sb")
                        nc.scalar.copy(sb[:], ps[:])
                        eng = nc.sync
                        eng.dma_start(
                            out=out_v[:, q * 8 : (q + 1) * 8, gsl],
                            in_=sb[:].rearrange("p (j g) -> p j g", j=8),
                        )
    # Bacc defers register allocation + wait-splitting to compile(); the
    # bass2jax/PJRT path serializes BIR directly, so run it here.
    nc.compile()
    return nc


def _get_program(reps=1, use_i16=USE_I16):
    key = (reps, use_i16)
    if key not in _PROGRAMS:
        _PROGRAMS[key] = _build_program(reps, use_i16)
    return _PROGRAMS[key]


def _host_prep(x, weights, idx_a, idx_b, use_i16=USE_I16):
    x = np.asarray(x, dtype=np.float32)
    if use_i16:
        u = np.clip(np.rint((x.astype(np.float64) - 0.5) * 65536.0), -32768, 32767)
        xt = np.ascontiguousarray(u.astype(np.int16).T)
    else:
        xt = np.ascontiguousarray(x.T)

    # truth table: T[i, j] = bit (3-j) of i
    tbl = ((np.arange(16)[:, None] >> (3 - np.arange(4))[None, :]) & 1).astype(
        np.float64
    )
    w = np.asarray(weights, dtype=np.float64)
    w = w - w.max(axis=-1, keepdims=True)
    e = np.exp(w)
    p = e / e.sum(axis=-1, keepdims=True)
    c = p @ tbl  # [O, 4]
    k0 = c[:, 0]
    ka = c[:, 2] - c[:, 0]
    kb = c[:, 1] - c[:, 0]
    kab = c[:, 0] - c[:, 1] - c[:, 2] + c[:, 3]
    if use_i16:
        al = 1.0 / 65536.0
        K0 = k0 + ka / 2 + kb / 2 + kab / 4
        KA = al * (ka + kab / 2)
        KB = al * (kb + kab / 2)
        KAB = al * al * kab
    else:
        K0, KA, KB, KAB = k0, ka, kb, kab
    coef = np.stack([K0, KA, KB, KAB], axis=1).astype(np.float32)  # [O, 4]

    ia = np.asarray(idx_a, dtype=np.int32)
    ib = np.asarray(idx_b, dtype=np.int32)
    return xt, coef, ia, ib


def make_in_maps(x, weights, idx_a, idx_b, use_i16=USE_I16):
    xt, coef, ia, ib = _host_prep(x, weights, idx_a, idx_b, use_i16)
    in_maps = []
    for k in range(NCORES):
        osl = slice(k * OSH, (k + 1) * OSH)
        # swizzle: gate g (within shard) = gb*128 + p  ->  [p, gb]
        ia_k = np.ascontiguousarray(ia[osl].reshape(GBLOCKS, P).T)
        ib_k = np.ascontiguousarray(ib[osl].reshape(GBLOCKS, P).T)
        # coef: [GBLOCKS, P, 4] -> [P, GBLOCKS, 4] -> [P, GBLOCKS*4]
        coef_k = np.ascontiguousarray(
            coef[osl].reshape(GBLOCKS, P, 4).transpose(1, 0, 2).reshape(P, GBLOCKS * 4)
        )
        in_maps.append({"xt": xt, "idxa": ia_k, "idxb": ib_k, "coef": coef_k})
    return in_maps


def run_kernel(x, weights, idx_a, idx_b, trace=False, use_i16=USE_I16):
    """Returns (out, BassKernelResults)."""
    in_maps = make_in_maps(x, weights, idx_a, idx_b, use_i16)
    nc = _get_program(1, use_i16)
    try:
        res = run_bass_kernel_spmd(nc, in_maps, list(range(NCORES)), trace=trace)
    except Exception:
        # transient device/tunnel hiccups (e.g. NRT_EXEC_UNIT_UNRECOVERABLE)
        # have been observed once; one retry is cheap insurance.
        res = run_bass_kernel_spmd(nc, in_maps, list(range(NCORES)), trace=trace)
    out = np.concatenate([res.results[k]["out"] for k in range(NCORES)], axis=1)
    return out, res


def kernel(x, weights, idx_a, idx_b):
    x = np.asarray(x, dtype=np.float32)
    # int16 fixed-point encoding assumes x in [0, 1] (the spec'd fill);
    # fall back to the exact f32 path for any other range.
    use_i16 = USE_I16 and x.min() >= 0.0 and x.max() <= 1.0
    out, _ = run_kernel(x, weights, idx_a, idx_b, trace=False, use_i16=use_i16)
    return out
